# revision 1
# baseline (speedup 1.0000x reference)
"""Trainium2 Bass kernel for nn_ConvEnhanced (conv/attn/quantum fused head).

Reference math per sample (x is (16,) f32, all in [0,1)):
    cls  = sigmoid(dot(x, w) + b)
    attn = mean_j sigmoid(a * x_j)
    q    = mean_j sin^2(pi * x_j / 2)        (the threshold/where is a no-op for x >= 0)
    out  = alpha * cls * attn + (1 - alpha) * q

Device strategy (pure data parallel over 8 cores, 524288 samples/core):
  - x is cast to fp16 on the host (|rel err| <= 2^-11, far inside the 2e-2
    gate) and shipped as (128, 65536): partition p owns 4096 samples packed
    per device tile as [n_tiles, KE, t_tile] (element-major within a tile) so
    every matmul rhs below is a CONTIGUOUS 1KB run per partition -- a strided
    (j::16) rhs measures ~5x slower PE streaming. Pure-copy HWDGE DMA at half
    the f32 byte count.
  - ScalarE does the two transcendental passes (1 elem/cycle/lane, dtype
    independent -> this engine is the ~110us roofline of the kernel):
        th = tanh((a/2) * x)        [sigmoid(ax) = 0.5 + 0.5*tanh(ax/2)]
        cs = sin(pi/2 - pi*x)       [= cos(pi*x); sin^2(pi x/2) = (1-cos(pi x))/2]
    Tanh and Sin share one ACT table set (silu_and_others) -> single load.
  - TensorE does the per-sample segmented sums (16 elems along the free dim)
    as 16 PSUM-accumulating N=512 matmuls per reduction with stride-16 APs:
        S_wx  += diag(w_j) @ x[:, j::16]     (fp16 in, fp32 accum)
        S_th  += I @ th[:, j::16]
        S_cs  += I @ cs[:, j::16]
    fp16 weights get FWL, so LDWEIGHTS (~53ns) hides behind each 215ns
    matmul via the PE's background weight buffer.
  - ScalarE: t_c = tanh(0.5*S_wx + b/2)  ->  cls = 0.5*(1 + t_c)
  - VectorE tail combine:
        out = c0 + c1*t_c + c2*(S_th + t_c*S_th) + c3*S_cs
        c0 = alpha/4 + (1-alpha)/2, c1 = alpha/4, c2 = alpha/64, c3 = -(1-alpha)/32
"""

import numpy as np

try:
    import concourse.bass as bass  # noqa: F401
except ImportError:  # pragma: no cover
    import sys

    sys.path.insert(0, "/opt/trn_rl_repo")
    import concourse.bass as bass  # noqa: F401

B = 4_194_304  # total samples
N_CORES = 8
P = 128  # partitions
KE = 16  # elements per sample (4x4 patch)
B_LOC = B // N_CORES  # samples per core
SPP = B_LOC // P  # samples per partition (4096)

_NC_CACHE = {}

# Per-device-tile sample counts (per partition). Small edge tiles shrink the
# pipeline fill (first x DMA) and drain (last tile's MM+tail chain); 512-deep
# middle tiles amortize ACT/MM instruction overhead. Sum must equal SPP.
TILES = (192, 512, 512, 512, 512, 512, 512, 512, 192, 128)

# j-blocks of the cos pass computed on ScalarE (the rest go to an idle-DVE
# degree-5 odd polynomial: cos(pi x) = sin(w), w = pi/2 - pi*x, |w| <= pi/2,
# sin(w) ~ w*(A0 + A1 w^2 + A2 w^4), minimax abs err 5.2e-4).
K_SPLIT = 10
SIN_A0 = 0.9999875
SIN_A1 = -0.16632039
SIN_A2 = 0.00777582
SIN_S1 = SIN_A1 / SIN_A2  # (v + S1)*v factorization for the STT Horner step


def _build(spp, tiles):
    """Build the Bass/Tile program for one core (SPMD: identical on all cores).

    spp:   samples per partition held by this core
    tiles: per-iteration sample counts (per partition), summing to spp
    """
    import concourse.bacc as bacc
    import concourse.bass as bass
    import concourse.tile as tile
    from concourse import mybir

    F32 = mybir.dt.float32
    F16 = mybir.dt.float16
    A = mybir.ActivationFunctionType
    Op = mybir.AluOpType

    tiles = list(tiles)
    assert sum(tiles) == spp
    t_max = max(tiles)
    ft_max = KE * t_max

    nc = bacc.Bacc("TRN2", target_bir_lowering=False)
    x_d = nc.declare_dram_parameter("x", [P, spp * KE], F16, isOutput=False)
    wd_d = nc.declare_dram_parameter("wdiag", [P, KE * P], F16, isOutput=False)
    id_d = nc.declare_dram_parameter("ident", [P, P], F16, isOutput=False)
    c_d = nc.declare_dram_parameter("consts", [P, 8], F32, isOutput=False)
    o_d = nc.declare_dram_parameter("out", [P, spp], F32, isOutput=True)

    PI = float(np.pi)

    with tile.TileContext(nc) as tc:
        with (
            tc.tile_pool(name="const", bufs=1) as cpool,
            tc.tile_pool(name="xp", bufs=3) as xpool,
            tc.tile_pool(name="actp", bufs=2) as apool,
            tc.tile_pool(name="smallp", bufs=2) as spool,
            tc.tile_pool(name="dvep", bufs=2) as dpool,
            tc.tile_pool(name="psump", bufs=2, space="PSUM") as ppool,
        ):
            # Consts stay on the sync HWDGE queue (the gpsimd SWDGE path
            # measures ~7.6us for these 548KB -- it would gate the first
            # matmul). Queue order: c, tile-0 x (the first-ACT critical
            # path), then wd/id, which are only needed once matmuls start.
            c_sb = cpool.tile([P, 8], F32, tag="c")
            nc.sync.dma_start(c_sb[:], c_d[:])
            x_first = xpool.tile([P, ft_max], F16, tag="x")
            nc.sync.dma_start(
                x_first[:, 0 : KE * tiles[0]], x_d[:, 0 : KE * tiles[0]]
            )
            wd_sb = cpool.tile([P, KE * P], F16, tag="wd")
            nc.sync.dma_start(wd_sb[:], wd_d[:])
            id_sb = cpool.tile([P, P], F16, tag="id")
            nc.sync.dma_start(id_sb[:], id_d[:])

            # Dummy 1-element ACT: forces the (single, pinned) ACT table set
            # to load while the tile-0 x DMA is still in flight, instead of
            # serializing ~1.3us of table load after it lands.
            warm_i = cpool.tile([P, 1], F32, tag="warm_i")
            nc.gpsimd.memset(warm_i[:], 0.0)
            warm_o = cpool.tile([P, 1], F32, tag="warm_o")
            nc.scalar.activation(warm_o[:], warm_i[:], A.Tanh)

            wd_v = wd_sb[:].rearrange("p (j m) -> p j m", j=KE)

            m_max = (KE - K_SPLIT) * t_max  # DVE-computed cos elems per tile

            def emit_tail(st):
                """Combine + store for a finished tile (pipelined one tile
                behind the MMs so the DVE never head-of-line blocks)."""
                t_tile, off, ps_wx, ps_th, ps_cs, tc_t = st
                # m1 = t_c * S_th ; a1 = S_th + m1
                m1 = spool.tile([P, t_max], F32, tag="m1")
                nc.vector.tensor_mul(m1[:, 0:t_tile], tc_t[:, 0:t_tile], ps_th[:, 0:t_tile])
                a1 = spool.tile([P, t_max], F32, tag="a1")
                nc.vector.tensor_add(a1[:, 0:t_tile], m1[:, 0:t_tile], ps_th[:, 0:t_tile])
                # t1 = c1*t_c + c0  (stays on DVE: gpsimd compute shares the
                # DVE's SBUF port -- offloading there measured DVE 96->115us)
                t1 = spool.tile([P, t_max], F32, tag="t1")
                nc.vector.tensor_scalar(
                    t1[:, 0:t_tile], tc_t[:, 0:t_tile],
                    c_sb[:, 2:3], c_sb[:, 3:4], Op.mult, Op.add,
                )
                # p1 = c3*S_cs + t1
                p1 = spool.tile([P, t_max], F32, tag="p1")
                nc.vector.scalar_tensor_tensor(
                    p1[:, 0:t_tile], ps_cs[:, 0:t_tile], c_sb[:, 5:6],
                    t1[:, 0:t_tile], Op.mult, Op.add,
                )
                # out = c2*a1 + p1
                o_t = spool.tile([P, t_max], F32, tag="o")
                nc.vector.scalar_tensor_tensor(
                    o_t[:, 0:t_tile], a1[:, 0:t_tile], c_sb[:, 4:5],
                    p1[:, 0:t_tile], Op.mult, Op.add,
                )
                nc.sync.dma_start(o_d[:, off : off + t_tile], o_t[:, 0:t_tile])

            pending = None
            off = 0
            for t_idx, t_tile in enumerate(tiles):
                ft = KE * t_tile
                ks = K_SPLIT * t_tile  # first ks elems: cos on ScalarE
                m = ft - ks  # rest: cos on DVE
                e0 = off * KE  # element offset of this tile in DRAM
                if t_idx == 0:
                    x_t = x_first  # prefetched above, ahead of wd/id
                else:
                    x_t = xpool.tile([P, ft_max], F16, tag="x")
                    nc.sync.dma_start(x_t[:, 0:ft], x_d[:, e0 : e0 + ft])

                # th = tanh((a/2) x), cs = sin(pi/2 - pi x) = cos(pi x)
                th_t = apool.tile([P, ft_max], F16, tag="th")
                nc.scalar.activation(
                    th_t[:, 0:ft], x_t[:, 0:ft], A.Tanh, scale=c_sb[:, 0:1]
                )
                cs_t = apool.tile([P, K_SPLIT * t_max], F16, tag="cs")
                nc.scalar.activation(
                    cs_t[:, 0:ks], x_t[:, 0:ks], A.Sin, bias=c_sb[:, 6:7], scale=-PI
                )
                # DVE polynomial for the last KE-K_SPLIT j-blocks:
                # w = pi/2 - pi*x; v = w^2; cs2 = w*(A2*((v+S1)*v) + A0)
                w_t = dpool.tile([P, m_max], F16, tag="w")
                nc.vector.tensor_scalar(
                    w_t[:, 0:m], x_t[:, ks:ft], -PI, PI / 2.0, Op.mult, Op.add
                )
                v_t = dpool.tile([P, m_max], F16, tag="v")
                nc.vector.tensor_mul(v_t[:, 0:m], w_t[:, 0:m], w_t[:, 0:m])
                h_t = dpool.tile([P, m_max], F16, tag="h")
                nc.vector.scalar_tensor_tensor(
                    h_t[:, 0:m], v_t[:, 0:m], SIN_S1, v_t[:, 0:m], Op.add, Op.mult
                )
                q_t = dpool.tile([P, m_max], F16, tag="q")
                nc.vector.tensor_scalar(
                    q_t[:, 0:m], h_t[:, 0:m], SIN_A2, SIN_A0, Op.mult, Op.add
                )
                cs2_t = dpool.tile([P, m_max], F16, tag="cs2")
                nc.vector.tensor_mul(cs2_t[:, 0:m], q_t[:, 0:m], w_t[:, 0:m])

                ps_wx = ppool.tile([P, t_max], F32, tag="pwx")
                ps_th = ppool.tile([P, t_max], F32, tag="pth")
                ps_cs = ppool.tile([P, t_max], F32, tag="pcs")

                # tile-packed layout: columns [j*t_tile, (j+1)*t_tile) hold
                # element j of every sample in the tile -> contiguous rhs
                for j in range(KE):
                    nc.tensor.matmul(
                        ps_wx[:, 0:t_tile],
                        lhsT=wd_v[:, j, :],
                        rhs=x_t[:, bass.ts(j, t_tile)],
                        start=(j == 0),
                        stop=(j == KE - 1),
                    )
                for j in range(KE):
                    nc.tensor.matmul(
                        ps_th[:, 0:t_tile],
                        lhsT=id_sb[:],
                        rhs=th_t[:, bass.ts(j, t_tile)],
                        start=(j == 0),
                        stop=(j == KE - 1),
                    )
                for j in range(KE):
                    rhs = (
                        cs_t[:, bass.ts(j, t_tile)]
                        if j < K_SPLIT
                        else cs2_t[:, bass.ts(j - K_SPLIT, t_tile)]
                    )
                    nc.tensor.matmul(
                        ps_cs[:, 0:t_tile],
                        lhsT=id_sb[:],
                        rhs=rhs,
                        start=(j == 0),
                        stop=(j == KE - 1),
                    )

                # t_c = tanh(0.5*S_wx + b/2); cls = 0.5*(1+t_c)
                tc_t = spool.tile([P, t_max], F32, tag="tc")
                nc.scalar.activation(
                    tc_t[:, 0:t_tile], ps_wx[:, 0:t_tile], A.Tanh,
                    bias=c_sb[:, 1:2], scale=0.5,
                )
                if pending is not None:
                    emit_tail(pending)
                pending = (t_tile, off, ps_wx, ps_th, ps_cs, tc_t)
                off += t_tile
            emit_tail(pending)

    # Pin Tanh+Sin to the one table set that holds both (silu_and_others) so
    # the act-table pass emits a single load instead of flip-flopping between
    # trig_and_small and exp_and_others every tile (~2.7us per switch on the
    # bottleneck engine). Indices/order of the table dict are preserved, so
    # act_func_set_id stays consistent with act_info.json.
    import concourse.hw_specs as hw_specs

    _orig_gat = hw_specs.get_activation_tables

    def _pinned_tables(arch):
        tabs = {k: set(v) for k, v in _orig_gat(arch).items()}
        assert A.Tanh in tabs["silu_and_others"] and A.Sin in tabs["silu_and_others"]
        for name, fns in tabs.items():
            if name != "silu_and_others":
                fns.discard(A.Tanh)
                fns.discard(A.Sin)
        return tabs

    bacc.get_activation_tables = _pinned_tables
    try:
        nc.compile()
    finally:
        bacc.get_activation_tables = _orig_gat
    return nc


def get_nc(spp=SPP, tiles=None):
    if tiles is None:
        tiles = TILES
    key = (spp, tuple(tiles))
    if key not in _NC_CACHE:
        _NC_CACHE[key] = _build(spp, tiles)
    return _NC_CACHE[key]


def make_const_inputs(conv_w, conv_b, attn_w, alpha):
    """Host-side packing of the tiny runtime parameters into device tensors."""
    w = np.asarray(conv_w, dtype=np.float32).reshape(KE)
    b = float(np.asarray(conv_b, dtype=np.float32).reshape(-1)[0])
    a = float(np.asarray(attn_w, dtype=np.float32).reshape(-1)[0])
    al = float(np.asarray(alpha, dtype=np.float32))

    wdiag = np.zeros((P, KE, P), dtype=np.float16)
    idx = np.arange(P)
    wdiag[idx, :, idx] = w[None, :].astype(np.float16)
    wdiag = np.ascontiguousarray(wdiag.reshape(P, KE * P))

    ident = np.ascontiguousarray(np.eye(P, dtype=np.float16))

    row = np.zeros(8, dtype=np.float32)
    row[0] = a / 2.0  # scale for tanh(a x / 2)
    row[1] = b / 2.0  # bias for tanh(0.5 S_wx + b/2)
    row[2] = al / 4.0  # c1
    row[3] = al / 4.0 + (1.0 - al) / 2.0  # c0
    row[4] = al / 64.0  # c2
    row[5] = -(1.0 - al) / 32.0  # c3
    row[6] = np.pi / 2.0  # bias for sin(pi/2 - pi x) = cos(pi x)
    consts = np.ascontiguousarray(np.tile(row[None, :], (P, 1)))
    return wdiag, ident, consts


def pack_x(x3d, tiles):
    """[..., spp, KE] f32 -> [..., spp*KE] fp16, tile-packed element-major.

    Within each device tile of t samples, all t element-0 values come first,
    then element-1, ... so each matmul rhs is a contiguous run.
    """
    *lead, spp, ke = x3d.shape
    assert sum(tiles) == spp
    v = x3d.astype(np.float16)
    out = np.empty((*lead, spp * ke), dtype=np.float16)
    off = 0
    for t in tiles:
        seg = np.swapaxes(v[..., off : off + t, :], -1, -2)
        out[..., off * ke : (off + t) * ke] = seg.reshape(*lead, t * ke)
        off += t
    return out


def prep_x(x, tiles=TILES):
    """Cast the full f32 input to fp16, shard and tile-pack (cores, P, spp*KE)."""
    x = np.asarray(x)
    assert x.size == B * KE
    return pack_x(x.reshape(N_CORES, P, SPP, KE), tiles)


def kernel(x, conv_w, conv_b, attn_w, alpha):
    from concourse.bass_utils import run_bass_kernel_spmd

    xs = prep_x(x)
    wdiag, ident, consts = make_const_inputs(conv_w, conv_b, attn_w, alpha)

    nc = get_nc()
    in_maps = [
        {"x": xs[c], "wdiag": wdiag, "ident": ident, "consts": consts}
        for c in range(N_CORES)
    ]
    res = run_bass_kernel_spmd(nc, in_maps, list(range(N_CORES)))
    out = np.concatenate(
        [np.asarray(res.results[c]["out"], dtype=np.float32).reshape(-1) for c in range(N_CORES)]
    )
    return out



# revision 6
# speedup vs baseline: 1.0966x; 1.0966x over previous
"""Trainium2 Bass kernel for nn_ConvEnhanced (conv/attn/quantum fused head).

Reference math per sample (x is (16,) f32, all in [0,1)):
    cls  = sigmoid(dot(x, w) + b)
    attn = mean_j sigmoid(a * x_j)
    q    = mean_j sin^2(pi * x_j / 2)        (threshold/where is a no-op, x >= 0)
    out  = alpha * cls * attn + (1 - alpha) * q

Device strategy (pure data parallel over 8 cores, 524288 samples/core):
  - x cast to fp16 on host (rel err <= 2^-11, inside the 2e-2 gate), shipped
    as (128, 65536) tile-packed j-major: within a device tile of t samples,
    j-block j is a contiguous t-run (matmul rhs stays contiguous; strided
    rhs measures ~5x slower PE streaming).
  - ScalarE: one full tanh pass th = tanh((a/2)x) (sigmoid(ax) =
    .5+.5tanh(ax/2)); a SMALL Sin pass (K_ACT j-blocks) cs = sin(pi/2-pi*x)
    = cos(pi*x); and the cls tanh on the dot-product sums.
  - DVE: the quantum path for the remaining 16-K_ACT j-blocks runs as ONE
    fused custom-DVE op per j-block (8-deep ALU pipeline, ~1.1 cyc/elem,
    validated on HW):
        SINSQ_INIT: qacc  = P5(x)^2          (j = first DVE block)
        SINSQ_ACC:  qacc += P5(x)^2          (remaining blocks, in-place)
    where P5(x) = x*(B0 + B1 x^2 + B2 x^4) ~ sin((pi/2)x), coefficients
    minimax-fit on |P5^2 - sin^2| (max err 9.8e-5). This replaces both the
    5-op elementwise polynomial AND the PE reduction of the baseline: qacc
    accumulates per-sample sums directly.
  - TensorE: segmented sums as PSUM-accumulating N=t matmuls per j-block:
        S_wx  += diag(w_j) @ x[:, j-block]     (fp16 in, fp32 accum)
        S_th  += I @ th[:, j-block]            (16 blocks)
        S_cs  += I @ cs[:, j-block]            (K_ACT blocks)
  - Tail (DVE, pipelined one tile behind): with tc = tanh(.5 S_wx + b/2),
        out = (1+tc)*(alpha/4 + alpha/64*S_th)            [custom TAIL1]
            + (-(1-alpha)/32)*S_cs + ...                  [stt]
            + ((1-alpha)/16)*qacc + const                 [AFFINE_THEN_ADD]
    out is written fp16 (values in (0,1)) and upcast on the host.
"""

import numpy as np

try:
    import concourse.bass as bass  # noqa: F401
except ImportError:  # pragma: no cover
    import sys

    sys.path.insert(0, "/opt/trn_rl_repo")
    import concourse.bass as bass  # noqa: F401

import concourse.dve_ops as dve_ops
from concourse.dve_ops import DveOp
from concourse.dve_spec import (
    C0,
    C1,
    C2,
    One,
    Spec,
    Src0,
    Src1,
    lower as dve_lower,
    sq,
)
from concourse.dve_spec import _has_src1 as has_src1
from concourse.dve_uop import DveOpSpec

B = 4_194_304  # total samples
N_CORES = 8
P = 128  # partitions
KE = 16  # elements per sample (4x4 patch)
B_LOC = B // N_CORES  # samples per core
SPP = B_LOC // P  # samples per partition (4096)

# j-blocks evaluated on ScalarE's Sin table (cos(pi x) values, PE-reduced);
# the other KE-K_ACT j-blocks run on the fused DVE sin^2 accumulator ops.
K_ACT = 3

# sin((pi/2)x) ~ x*(B0 + B1 x^2 + B2 x^4): minimax fit of |P^2 - sin^2|
# over x in [0,1], max err 9.8e-5.
SIN_B0 = 1.57009095
SIN_B1 = -0.64138591
SIN_B2 = 0.07134415

# Per-device-tile sample counts (per partition). Small edge tiles shrink
# pipeline fill/drain; 1024-deep middle tiles amortize per-instruction
# overhead (custom-DVE op carries ~130ns fixed cost). Sum must equal SPP.
TILES = (256, 1024, 1024, 1024, 512, 256)

_NC_CACHE = {}


def _register_op(name, spec, subdim=False):
    """Register a custom DVE op into the dve_ops tables (idempotent)."""
    if name in dve_ops._SUB_OPCODE_FOR_NAME:
        return next(o for o in dve_ops.OPS if o.name == name)
    row = dve_ops._CUSTOM_DVE_ROW_BASE + len(dve_ops.OPS)
    assert row < 0x20, "custom-DVE opcode rows exhausted"
    shas = {}
    for ver in ("v3", "v4"):
        so = DveOpSpec(
            name=name, opcode=row, uops=dve_lower(spec, ver=ver),
            rd1_en=has_src1(spec),
        )
        shas[ver] = so.sha(ver)
    op = DveOp(name, spec, subdim=subdim, uops_sha=shas)
    dve_ops.OPS.append(op)
    dve_ops._SUB_OPCODE_FOR_NAME[name] = row
    dve_ops.CUSTOM_DVE_SPECS[name] = spec
    return op


def _p5sq(x):
    v = x * x
    p = ((v * SIN_B2 + SIN_B1) * v + SIN_B0) * x
    return p * p


def _sinsq_init_ref(in0, in1, c0, c1, c2):
    return _p5sq(in0.astype(np.float32)).astype(np.float32)


def _sinsq_acc_ref(in0, in1, c0, c1, c2):
    return (_p5sq(in0.astype(np.float32)) + in1.astype(np.float32)).astype(
        np.float32
    )


def _tail1_ref(in0, in1, c0, c1, c2):
    # (tc + 1) * (S_th * c0 + c1)
    return (
        (in0.astype(np.float32) + 1.0) * (in1.astype(np.float32) * c0 + c1)
    ).astype(np.float32)


_v = sq(Src0)
_p5 = ((_v * C0 + C1) * _v + C2) * Src0
SINSQ_INIT = _register_op(
    "NNCE_SINSQ_INIT", Spec(body=sq(_p5), reference=_sinsq_init_ref)
)
SINSQ_ACC = _register_op(
    "NNCE_SINSQ_ACC", Spec(body=sq(_p5) + Src1, reference=_sinsq_acc_ref)
)
TAIL1 = _register_op(
    "NNCE_TAIL1",
    Spec(body=(Src0 + One) * (Src1 * C0 + C1), reference=_tail1_ref),
)


def _build(spp, tiles, k_act=K_ACT):
    """Build the Bass/Tile program for one core (SPMD: identical on all)."""
    import concourse.bacc as bacc
    import concourse.tile as tile
    from concourse import mybir

    F32 = mybir.dt.float32
    F16 = mybir.dt.float16
    A = mybir.ActivationFunctionType
    Op = mybir.AluOpType

    tiles = list(tiles)
    assert sum(tiles) == spp
    t_max = max(tiles)
    ft_max = KE * t_max
    k_dve = KE - k_act  # j-blocks on the DVE sin^2 path (0..k_dve-1)
    # PE/PSUM granularity: a matmul dest must fit one PSUM bank (512 f32).
    H = 512
    assert all(t % H == 0 or t <= H for t in tiles)

    PI = float(np.pi)

    nc = bacc.Bacc("TRN2", target_bir_lowering=False)
    x_d = nc.declare_dram_parameter("x", [P, spp * KE], F16, isOutput=False)
    wd_d = nc.declare_dram_parameter("wdiag", [P, KE * P], F16, isOutput=False)
    id_d = nc.declare_dram_parameter("ident", [P, P], F16, isOutput=False)
    c_d = nc.declare_dram_parameter("consts", [P, 12], F32, isOutput=False)
    o_d = nc.declare_dram_parameter("out", [P, spp], F16, isOutput=True)

    with tile.TileContext(nc) as tc:
        with (
            tc.tile_pool(name="const", bufs=1) as cpool,
            tc.tile_pool(name="xp", bufs=2) as xpool,
            tc.tile_pool(name="thp", bufs=2) as thpool,
            tc.tile_pool(name="csp", bufs=2) as cspool,
            tc.tile_pool(name="qp", bufs=2) as qpool,
            tc.tile_pool(name="tcp", bufs=2) as tcpool,
            tc.tile_pool(name="tlp", bufs=2) as tlpool,
            tc.tile_pool(name="op", bufs=2) as opool,
            tc.tile_pool(name="pwx", bufs=2, space="PSUM") as wxpool,
            tc.tile_pool(name="pth", bufs=2, space="PSUM") as thppool,
            tc.tile_pool(name="pcs", bufs=2, space="PSUM") as csppool,
        ):
            # Consts ride the sync HWDGE queue. Order: consts, tile-0 x
            # (first-ACT critical path), then wd/id (needed once mms start).
            c_sb = cpool.tile([P, 12], F32, tag="c")
            nc.sync.dma_start(c_sb[:], c_d[:])
            x_first = xpool.tile([P, ft_max], F16, tag="x")
            nc.sync.dma_start(
                x_first[:, 0 : KE * tiles[0]], x_d[:, 0 : KE * tiles[0]]
            )
            wd_sb = cpool.tile([P, KE * P], F16, tag="wd")
            nc.sync.dma_start(wd_sb[:], wd_d[:])
            id_sb = cpool.tile([P, P], F16, tag="id")
            nc.sync.dma_start(id_sb[:], id_d[:])

            # Dummy 1-elem ACT: force the single pinned table set (holds
            # both Tanh and Sin) to load behind the tile-0 x DMA.
            warm_i = cpool.tile([P, 1], F32, tag="warm_i")
            nc.gpsimd.memset(warm_i[:], 0.0)
            warm_o = cpool.tile([P, 1], F32, tag="warm_o")
            nc.scalar.activation(warm_o[:], warm_i[:], A.Tanh)

            wd_v = wd_sb[:].rearrange("p (j m) -> p j m", j=KE)

            def emit_tail(st):
                """Combine + store for a finished 512-half (one unit behind)."""
                h_len, off, ps_th, ps_cs, qacc, q0, tc_t = st
                # p1 = (tc+1) * (c_thm*S_th + c_tha)
                p1 = tlpool.tile([P, H], F32, tag="p1")
                nc.vector._custom_dve(
                    TAIL1,
                    out=p1[:, 0:h_len],
                    in0=tc_t[:, 0:h_len],
                    in1=ps_th[:, 0:h_len],
                    s0=c_sb[:, 2:3],
                    s1=c_sb[:, 3:4],
                )
                if k_act > 0:
                    # p2 = c_cs*S_cs + p1
                    p2 = tlpool.tile([P, H], F32, tag="p2")
                    nc.vector.scalar_tensor_tensor(
                        p2[:, 0:h_len], ps_cs[:, 0:h_len], c_sb[:, 4:5],
                        p1[:, 0:h_len], Op.mult, Op.add,
                    )
                else:
                    p2 = p1
                # out = (c_q*qacc + c_0) + p2   (fp16 out)
                o_t = opool.tile([P, H], F16, tag="o")
                nc.vector.affine_then_add(
                    o_t[:, 0:h_len], qacc[:, q0 : q0 + h_len], p2[:, 0:h_len],
                    c_sb[:, 5:6], c_sb[:, 6:7],
                )
                nc.sync.dma_start(o_d[:, off : off + h_len], o_t[:, 0:h_len])

            pending = None
            off = 0
            for t_idx, t_tile in enumerate(tiles):
                ft = KE * t_tile
                e0 = off * KE
                if t_idx == 0:
                    x_t = x_first
                else:
                    x_t = xpool.tile([P, ft_max], F16, tag="x")
                    nc.sync.dma_start(x_t[:, 0:ft], x_d[:, e0 : e0 + ft])

                # th = tanh((a/2) x) over all KE j-blocks
                th_t = thpool.tile([P, ft_max], F16, tag="th")
                nc.scalar.activation(
                    th_t[:, 0:ft], x_t[:, 0:ft], A.Tanh, scale=c_sb[:, 0:1]
                )
                # cs = sin(pi/2 - pi x) = cos(pi x) on the last k_act blocks
                if k_act > 0:
                    cs_t = cspool.tile([P, k_act * t_max], F16, tag="cs")
                    nc.scalar.activation(
                        cs_t[:, 0 : k_act * t_tile],
                        x_t[:, k_dve * t_tile : ft],
                        A.Sin,
                        bias=c_sb[:, 7:8],
                        scale=-PI,
                    )

                # Fused DVE sin^2 accumulation over j-blocks 0..k_dve-1
                qacc = qpool.tile([P, t_max], F32, tag="qacc")
                nc.vector._custom_dve(
                    SINSQ_INIT,
                    out=qacc[:, 0:t_tile],
                    in0=x_t[:, 0:t_tile],
                    s0=SIN_B2, s1=SIN_B1, imm2=SIN_B0,
                )
                for j in range(1, k_dve):
                    nc.vector._custom_dve(
                        SINSQ_ACC,
                        out=qacc[:, 0:t_tile],
                        in0=x_t[:, bass.ts(j, t_tile)],
                        in1=qacc[:, 0:t_tile],
                        s0=SIN_B2, s1=SIN_B1, imm2=SIN_B0,
                    )

                # PE reductions + tc + tail per 512-column half (PSUM bank)
                for h0 in range(0, t_tile, H):
                    h_len = min(H, t_tile - h0)
                    ps_wx = wxpool.tile([P, H], F32, tag="pwx")
                    ps_th = thppool.tile([P, H], F32, tag="pth")
                    if k_act:
                        ps_cs = csppool.tile([P, H], F32, tag="pcs")
                    else:
                        ps_cs = None

                    for j in range(KE):
                        nc.tensor.matmul(
                            ps_wx[:, 0:h_len],
                            lhsT=wd_v[:, j, :],
                            rhs=x_t[:, j * t_tile + h0 : j * t_tile + h0 + h_len],
                            start=(j == 0),
                            stop=(j == KE - 1),
                        )
                    for j in range(KE):
                        nc.tensor.matmul(
                            ps_th[:, 0:h_len],
                            lhsT=id_sb[:],
                            rhs=th_t[:, j * t_tile + h0 : j * t_tile + h0 + h_len],
                            start=(j == 0),
                            stop=(j == KE - 1),
                        )
                    for j in range(k_act):
                        nc.tensor.matmul(
                            ps_cs[:, 0:h_len],
                            lhsT=id_sb[:],
                            rhs=cs_t[:, j * t_tile + h0 : j * t_tile + h0 + h_len],
                            start=(j == 0),
                            stop=(j == k_act - 1),
                        )

                    # tc = tanh(0.5*S_wx + b/2)
                    tc_t = tcpool.tile([P, H], F32, tag="tc")
                    nc.scalar.activation(
                        tc_t[:, 0:h_len], ps_wx[:, 0:h_len], A.Tanh,
                        bias=c_sb[:, 1:2], scale=0.5,
                    )
                    if pending is not None:
                        emit_tail(pending)
                    pending = (h_len, off + h0, ps_th, ps_cs, qacc, h0, tc_t)
                off += t_tile
            emit_tail(pending)

    # Pin Tanh+Sin to the one table set holding both (silu_and_others) so
    # the act-table pass emits a single load instead of per-tile switches.
    import concourse.bacc as bacc
    import concourse.hw_specs as hw_specs

    _orig_gat = hw_specs.get_activation_tables

    def _pinned_tables(arch):
        tabs = {k: set(v) for k, v in _orig_gat(arch).items()}
        assert A.Tanh in tabs["silu_and_others"] and A.Sin in tabs["silu_and_others"]
        for name, fns in tabs.items():
            if name != "silu_and_others":
                fns.discard(A.Tanh)
                fns.discard(A.Sin)
        return tabs

    bacc.get_activation_tables = _pinned_tables
    try:
        nc.compile()
    finally:
        bacc.get_activation_tables = _orig_gat
    return nc


def get_nc(spp=SPP, tiles=None):
    if tiles is None:
        tiles = TILES
    key = (spp, tuple(tiles))
    if key not in _NC_CACHE:
        _NC_CACHE[key] = _build(spp, tiles)
    return _NC_CACHE[key]


def make_const_inputs(conv_w, conv_b, attn_w, alpha):
    """Host-side packing of the tiny runtime parameters."""
    w = np.asarray(conv_w, dtype=np.float32).reshape(KE)
    b = float(np.asarray(conv_b, np.float32).reshape(-1)[0])
    a = float(np.asarray(attn_w, np.float32).reshape(-1)[0])
    al = float(np.asarray(alpha, np.float32))

    wdiag = np.zeros((P, KE, P), dtype=np.float16)
    idx = np.arange(P)
    wdiag[idx, :, idx] = w[None, :].astype(np.float16)
    wdiag = np.ascontiguousarray(wdiag.reshape(P, KE * P))

    ident = np.ascontiguousarray(np.eye(P, dtype=np.float16))

    row = np.zeros(12, dtype=np.float32)
    row[0] = a / 2.0  # scale for tanh(a x / 2)
    row[1] = b / 2.0  # bias for tanh(0.5 S_wx + b/2)
    row[2] = al / 64.0  # TAIL1 c0 (S_th coeff)
    row[3] = al / 4.0  # TAIL1 c1
    row[4] = -(1.0 - al) / 32.0  # S_cs coeff
    row[5] = (1.0 - al) / 16.0  # qacc coeff
    row[6] = (1.0 - al) / 32.0 * K_ACT  # constant from cos->sin^2 rewrite
    row[7] = np.pi / 2.0  # bias for sin(pi/2 - pi x)
    consts = np.ascontiguousarray(np.tile(row[None, :], (P, 1)))
    return wdiag, ident, consts


def pack_x(x3d, tiles):
    """[..., spp, KE] f32 -> [..., spp*KE] fp16, tile-packed element-major."""
    *lead, spp, ke = x3d.shape
    assert sum(tiles) == spp
    v = x3d.astype(np.float16)
    out = np.empty((*lead, spp * ke), dtype=np.float16)
    off = 0
    for t in tiles:
        seg = np.swapaxes(v[..., off : off + t, :], -1, -2)
        out[..., off * ke : (off + t) * ke] = seg.reshape(*lead, t * ke)
        off += t
    return out


def prep_x(x, tiles=None):
    """Cast the f32 input to fp16, shard and tile-pack (cores, P, spp*KE)."""
    if tiles is None:
        tiles = TILES
    x = np.asarray(x)
    assert x.size == B * KE
    return pack_x(x.reshape(N_CORES, P, SPP, KE), tiles)


def kernel(x, conv_w, conv_b, attn_w, alpha):
    from concourse.bass_utils import run_bass_kernel_spmd

    xs = prep_x(x)
    wdiag, ident, consts = make_const_inputs(conv_w, conv_b, attn_w, alpha)

    nc = get_nc()
    in_maps = [
        {"x": xs[c], "wdiag": wdiag, "ident": ident, "consts": consts}
        for c in range(N_CORES)
    ]
    res = run_bass_kernel_spmd(nc, in_maps, list(range(N_CORES)))
    out = np.concatenate(
        [
            np.asarray(res.results[c]["out"]).astype(np.float32).reshape(-1)
            for c in range(N_CORES)
        ]
    )
    return out
